# revision 7
# baseline (speedup 1.0000x reference)
"""Sigmoid-attention MHA kernel for 8 Trainium2 NeuronCores.

Problem: x[4,2048,512], W_q/W_k/W_v/W_o[512,512] (already scaled).
  Q = x@Wq.T, K = x@Wk.T, V = x@Wv.T split into 8 heads of depth 64
  attn = sigmoid(QK^T/sqrt(64) - log(2048));  out = (attn@V merged)@Wo.T

Sharding: core c handles batch b=c//2, head-group g=c%2 (4 heads each).
Each core computes a partial output projection over its 256 head-features;
host sums the two partials per batch.

On-chip layout (per core):
  xt   [128,8192]  x[b].T chunked   xt[p, 2048*kc+t] = x[b].T[128*kc+p, t]
  qt/kt (2 tiles [128,2048])        Q^T/K^T rows=features, cols=tokens
  v    (16 tiles [128,256])         V natural rows=tokens
  scores.T computed per (head-pair, 128-key-chunk, 512-query-chunk) into
  PSUM [128,1024] (two heads side by side), sigmoid on ScalarE (fused
  0.125 scale + -log(2048) bias) -> SBUF, then attn.T@V accumulated over
  key chunks into per-head PSUM tiles -> O^T tiles -> output projection.
"""

import os
import numpy as np

DEBUG = bool(int(os.environ.get("KERNEL_DEBUG", "0")))

B, S, D = 4, 2048, 512
NH, DEPTH = 8, 64
G = 2          # head groups (one per core pair)
GF = 256       # features per group
NEG_LOG_S = float(np.float32(-np.log(np.float32(S))))
INV_SQRT_DK = 0.125

_CACHE = {}


def _build_nc():
    import concourse.bacc as bacc
    import concourse.tile as tile
    from concourse import mybir

    f32 = mybir.dt.float32
    nc = bacc.Bacc("TRN2", target_bir_lowering=False, debug=False, num_devices=8)

    xt_d = nc.dram_tensor("xt", [128, 8192], f32, kind="ExternalInput").ap()
    wq_d = nc.dram_tensor("wq", [128, 1024], f32, kind="ExternalInput").ap()
    wk_d = nc.dram_tensor("wk", [128, 1024], f32, kind="ExternalInput").ap()
    wv_d = nc.dram_tensor("wv", [128, 1024], f32, kind="ExternalInput").ap()
    wo_d = nc.dram_tensor("wo", [128, 1024], f32, kind="ExternalInput").ap()
    out_d = nc.dram_tensor("out", [S, D], f32, kind="ExternalOutput").ap()
    dbg = {}
    if DEBUG:
        for nm in ("qt", "kt", "ot"):
            dbg[nm] = [nc.dram_tensor(f"dbg_{nm}{m}", [128, 2048], f32,
                                      kind="ExternalOutput").ap() for m in range(2)]
        dbg["v"] = [nc.dram_tensor(f"dbg_v{t}", [128, 256], f32,
                                   kind="ExternalOutput").ap() for t in range(16)]

    with tile.TileContext(nc) as tc:
        with (
            tc.tile_pool(name="persist", bufs=1) as persist,
            tc.tile_pool(name="attn", bufs=3) as apool,
            tc.tile_pool(name="stage", bufs=2) as stage,
            tc.tile_pool(name="spsum", bufs=2, space="PSUM") as spsum,
            tc.tile_pool(name="opsum", bufs=4, space="PSUM") as opsum,
        ):
            Sig = mybir.ActivationFunctionType.Sigmoid

            bias_t = persist.tile([128, 1], f32, tag="bias", name="bias_t")
            nc.vector.memset(bias_t[:], NEG_LOG_S)

            wq_sb = persist.tile([128, 1024], f32, tag="wq", name="wq_sb")
            wk_sb = persist.tile([128, 1024], f32, tag="wk", name="wk_sb")
            wv_sb = persist.tile([128, 1024], f32, tag="wv", name="wv_sb")
            wo_sb = persist.tile([128, 1024], f32, tag="wo", name="wo_sb")
            xt = [persist.tile([128, 2048], f32, tag=f"xt{c}", name=f"xt{c}") for c in range(4)]
            nc.sync.dma_start(out=wq_sb[:], in_=wq_d[:])
            nc.sync.dma_start(out=wk_sb[:], in_=wk_d[:])
            for c in range(4):
                nc.sync.dma_start(out=xt[c][:], in_=xt_d[:, 2048 * c:2048 * (c + 1)])
            nc.sync.dma_start(out=wv_sb[:], in_=wv_d[:])
            nc.sync.dma_start(out=wo_sb[:], in_=wo_d[:])

            qt = [persist.tile([128, 2048], f32, tag=f"qt{m}", name=f"qt{m}") for m in range(2)]
            kt = [persist.tile([128, 2048], f32, tag=f"kt{m}", name=f"kt{m}") for m in range(2)]
            v = [persist.tile([128, 256], f32, tag=f"v{t}", name=f"v{t}") for t in range(16)]
            ot = [persist.tile([128, 2048], f32, tag=f"ot{m}", name=f"ot{m}") for m in range(2)]

            def proj_qk(w_sb, dst, mc, qc):
                # dst[:, 512qc:+512] = (W.T chunk).T @ x.T chunk
                ps = spsum.tile([128, 1024], f32, tag="s", name="ps")
                for kc in range(4):
                    nc.tensor.matmul(
                        ps[:, 0:512],
                        lhsT=w_sb[:, 256 * kc + 128 * mc:256 * kc + 128 * mc + 128],
                        rhs=xt[kc][:, 512 * qc:512 * (qc + 1)],
                        start=(kc == 0), stop=(kc == 3),
                    )
                nc.vector.tensor_copy(dst[:, 512 * qc:512 * (qc + 1)], ps[:, 0:512])

            def proj_v(tcks):
                for tck in tcks:
                    ps = spsum.tile([128, 1024], f32, tag="s", name="ps")
                    for kc in range(4):
                        nc.tensor.matmul(
                            ps[:, 0:256],
                            lhsT=xt[kc][:, 128 * tck:128 * (tck + 1)],
                            rhs=wv_sb[:, 256 * kc:256 * (kc + 1)],
                            start=(kc == 0), stop=(kc == 3),
                        )
                    nc.vector.tensor_copy(v[tck][:], ps[:, 0:256])

            def attention(p):
                # head pair p: heads (2p, 2p+1) of this group, features
                # [128p,128p+128) of qt/kt/ot, v columns [128p,128p+128)
                for qc in range(4):
                    psA = opsum.tile([128, 512], f32, tag="o", name="pso")
                    psB = opsum.tile([128, 512], f32, tag="o", name="pso")
                    qs = slice(512 * qc, 512 * (qc + 1))
                    for kc in range(16):
                        ks = slice(128 * kc, 128 * (kc + 1))
                        s = spsum.tile([128, 1024], f32, tag="s", name="ps")
                        nc.tensor.matmul(
                            s[:, 0:512],
                            lhsT=kt[p][0:64, ks], rhs=qt[p][0:64, qs],
                            start=True, stop=True,
                        )
                        nc.tensor.matmul(
                            s[:, 512:1024],
                            lhsT=kt[p][64:128, ks], rhs=qt[p][64:128, qs],
                            start=True, stop=True,
                        )
                        a = apool.tile([128, 1024], f32, tag="a", name="attn")
                        nc.scalar.activation(a[:], s[:], Sig,
                                             bias=bias_t[:], scale=INV_SQRT_DK)
                        nc.tensor.matmul(
                            psA[0:64, :],
                            lhsT=v[kc][:, 128 * p:128 * p + 64],
                            rhs=a[:, 0:512],
                            start=(kc == 0), stop=(kc == 15),
                        )
                        nc.tensor.matmul(
                            psB[64:128, :],
                            lhsT=v[kc][:, 128 * p + 64:128 * p + 128],
                            rhs=a[:, 512:1024],
                            start=(kc == 0), stop=(kc == 15),
                        )
                    nc.vector.tensor_copy(ot[p][0:64, qs], psA[0:64, :])
                    nc.vector.tensor_copy(ot[p][64:128, qs], psB[64:128, :])

            # Emission order defines both dependencies and scheduling
            # priority: producers must be emitted before consumers.
            for qc in range(4):
                proj_qk(wq_sb, qt[0], 0, qc)
            for qc in range(4):
                proj_qk(wk_sb, kt[0], 0, qc)
            proj_v(range(16))
            attention(0)
            for qc in range(4):
                proj_qk(wq_sb, qt[1], 1, qc)
            for qc in range(4):
                proj_qk(wk_sb, kt[1], 1, qc)
            attention(1)

            if DEBUG:
                for m in range(2):
                    nc.sync.dma_start(out=dbg["qt"][m], in_=qt[m][:])
                    nc.sync.dma_start(out=dbg["kt"][m], in_=kt[m][:])
                    nc.sync.dma_start(out=dbg["ot"][m], in_=ot[m][:])
                for t in range(16):
                    nc.sync.dma_start(out=dbg["v"][t], in_=v[t][:])

            # output projection: out[t, m] = sum_f O^T[f, t] * A[f, m]
            for sc in range(4):
                st = stage.tile([128, 4, 512], f32, tag="pstage", name="pstage")
                for ti in range(4):
                    tck = 4 * sc + ti
                    ps = spsum.tile([128, 1024], f32, tag="s", name="ps")
                    for c in range(2):
                        nc.tensor.matmul(
                            ps[:, 0:512],
                            lhsT=ot[c][:, 128 * tck:128 * (tck + 1)],
                            rhs=wo_sb[:, 512 * c:512 * (c + 1)],
                            start=(c == 0), stop=(c == 1),
                        )
                    nc.vector.tensor_copy(st[:, ti, :], ps[:, 0:512])
                dst = out_d[512 * sc:512 * (sc + 1), :].rearrange(
                    "(t p) m -> p t m", p=128)
                nc.sync.dma_start(out=dst, in_=st[:])

    nc.compile()
    return nc


def get_nc():
    if "nc" not in _CACHE:
        _CACHE["nc"] = _build_nc()
    return _CACHE["nc"]


def make_in_maps(x, W_q, W_k, W_v, W_o):
    x = np.ascontiguousarray(np.asarray(x, dtype=np.float32))
    ws = [np.asarray(w, dtype=np.float32) for w in (W_q, W_k, W_v, W_o)]
    W_q, W_k, W_v, W_o = ws

    def chunked(a, nchunks):
        # [128*nchunks, m] -> [128, nchunks*m] with chunk-major columns
        m = a.shape[1]
        return np.ascontiguousarray(
            a.reshape(nchunks, 128, m).transpose(1, 0, 2).reshape(128, nchunks * m))

    in_maps = []
    for c in range(8):
        b, g = divmod(c, 2)
        gf = slice(GF * g, GF * (g + 1))
        in_maps.append({
            "xt": chunked(np.ascontiguousarray(x[b].T), 4),
            "wq": chunked(np.ascontiguousarray(W_q[gf, :].T), 4),
            "wk": chunked(np.ascontiguousarray(W_k[gf, :].T), 4),
            "wv": chunked(np.ascontiguousarray(W_v[gf, :].T), 4),
            "wo": chunked(np.ascontiguousarray(W_o[:, gf].T), 2),
        })
    return in_maps


def kernel(x, W_q, W_k, W_v, W_o):
    from concourse.bass_utils import run_bass_kernel_spmd

    nc = get_nc()
    in_maps = make_in_maps(x, W_q, W_k, W_v, W_o)
    res = run_bass_kernel_spmd(nc, in_maps, list(range(8)))
    parts = [res.results[c]["out"] for c in range(8)]
    out = np.stack([parts[2 * b] + parts[2 * b + 1] for b in range(B)])
    return np.ascontiguousarray(out.astype(np.float32))
